# revision 13
# baseline (speedup 1.0000x reference)
"""Trainium2 Bass kernel: NeuralNearestNeighbors continuous-KNN weight volumes.

Reference computation (per row of D.reshape(b*m, o), K=8 rounds):
    logits = D / exp(log_temp)
    for k in range(K):
        w_k = log_softmax(logits);  out_k = exp(w_k)
        logits = logits + log1mexp(w_k)          # log(1 - p_k)
    W = stack(out_k, axis=-1)                     # (b, m, o, K)

Scale-invariant recurrence used on device: keep a state S that is an
arbitrary per-row scalar multiple of the round's softmax weights F_k,
with its true row-sum `a = sum(S)` tracked by the engines' accumulator.
Then F_k = S*(1/a) always (since sum(F_k) == 1), and

    S' = (F_k - 1) * F_k        a' = sum(S') = sum(F_k^2) - 1

reproduces the reference exactly:  F_{k+1} = (F_k - F_k^2)/(1 - sum F_k^2).
S stays in [-0.25, 0] after the first update, so fp16 state is safe
(all quantities carry *relative* fp16 error only).

Two chain variants:
  amr:    S' = (S*gam - 1)*S  via one custom-DVE affine_mul_reduce
          (gam = 1/a; equals (F-1)*F up to a positive-cancelling scale).
  native: Fc = S*gam (tensor_scalar), S' = (Fc-1)*Fc (scalar_tensor_tensor
          with accum) - two native DVE ops that can engage the 2-byte
          2x/4x DVE perf modes.

Per round each tile also writes F_k = S*gam into the k-strided slot of its
[P, O, K] output tile; those strided writes are spread over ACT/DVE/GpSimd
per a static pattern.  Recurrence rounds for a wave of 4 tiles share one
batched [P,4] reciprocal.  Waves are software-pipelined: the next wave's
exp's are emitted mid-wave so ACT never starves DVE at wave boundaries.

Output can be stored fp16 on device (halves HBM write traffic; rel err
~5e-4 << 2e-2 tolerance) and is upcast to f32 on the host.

Sharding: purely rowwise data-parallel over b*m = 16384 rows; 2048 rows
per core across 8 cores; log_temp replicated.
"""

import os

import numpy as np

B, M, O = 16, 1024, 512
K = 8
N_CORES = 8
ROWS = B * M                     # 16384
RPC = ROWS // N_CORES            # 2048 rows per core
P = 128
TILES = RPC // P                 # 16 row-tiles per core
WV = 4                           # tiles per wave
WAVES = TILES // WV

VARIANT = os.environ.get("KVAR", "h16a")

# variant -> (state_dtype, out_dtype, chain, pass1 engine counts (A, P, D))
_CFG = {
    "f32a": ("f32", "f32", "amr", (20, 6, 6)),
    "f32n": ("f32", "f32", "natr", (20, 6, 6)),
    "mx1": ("f32", "f32", "natr", (16, 0, 16)),
    "h16a": ("f16", "f16", "amr", (15, 13, 4)),
    "h16n": ("f16", "f16", "nat", (26, 0, 6)),
    "hyb": ("f16", "f32", "nat", (15, 0, 17)),
    "hybr": ("f16", "f32", "natr", (11, 0, 21)),
    "hybr2": ("f16", "f32", "natr", (23, 0, 9)),
    "hybr3": ("f16", "f32", "natr", (22, 0, 10)),
}

_cached = {}


def _make_pattern(n_act, n_pool, n_dve, total):
    """Largest-remainder round-robin spread of engine codes over slots."""
    pools = [("A", n_act), ("P", n_pool), ("D", n_dve)]
    credit = {c: 0.0 for c, _ in pools}
    out = []
    for _ in range(total):
        for c, n in pools:
            credit[c] += n / total
        pick = max(credit, key=lambda c: credit[c])
        credit[pick] -= 1.0
        out.append(pick)
    return out


def _build(variant):
    from contextlib import ExitStack

    import concourse.bacc as bacc
    import concourse.tile as tile
    from concourse import mybir

    f32 = mybir.dt.float32
    f16 = mybir.dt.float16
    Alu = mybir.AluOpType
    Act = mybir.ActivationFunctionType

    sdt_s, odt_s, chain, counts = _CFG[variant]
    sdt = f16 if sdt_s == "f16" else f32
    odt = f16 if odt_s == "f16" else f32
    pat = _make_pattern(*counts, total=K * WV)

    nc = bacc.Bacc(
        "TRN2",
        target_bir_lowering=False,
        debug=False,
        enable_asserts=False,
        num_devices=N_CORES,
    )
    d = nc.dram_tensor("d", [RPC, O], f32, kind="ExternalInput").ap()
    lt = nc.dram_tensor("log_temp", [1, 1], f32, kind="ExternalInput").ap()
    w = nc.dram_tensor("w", [RPC, O * K], odt, kind="ExternalOutput").ap()

    with tile.TileContext(nc) as tc, ExitStack() as ctx:
        singles = ctx.enter_context(tc.tile_pool(name="singles", bufs=1))
        dpool = ctx.enter_context(tc.tile_pool(name="dslab", bufs=1))
        gpool = ctx.enter_context(
            tc.tile_pool(name="state", bufs=28 if chain in ("nat", "natr") else 16)
        )
        outp = ctx.enter_context(
            tc.tile_pool(name="out", bufs=10 if odt == f16 else 7)
        )
        small = ctx.enter_context(tc.tile_pool(name="small", bufs=64))

        # log_temp -> 1/T = exp(-log_temp), replicated to all 128 partitions.
        lt_sb = singles.tile([P, 1], f32)
        nc.sync.dma_start(out=lt_sb[:, :], in_=lt.to_broadcast((P, 1)))
        invt = singles.tile([P, 1], f32)
        nc.scalar.activation(invt[:, :], lt_sb[:, :], Act.Exp, scale=-1.0)
        bias6 = singles.tile([P, 1], f32)
        nc.vector.memset(bias6[:, :], -6.0)

        din = d.rearrange("(t p) o -> p t o", p=P)
        dslab = dpool.tile([P, TILES, O], f32)
        for g in range(WAVES):
            # SWDGE path keeps the HWDGE rings free for output writes.
            nc.gpsimd.dma_start(
                out=dslab[:, g * WV : (g + 1) * WV, :],
                in_=din[:, g * WV : (g + 1) * WV, :],
            )

        wave_state = {}

        def emit_exps(g):
            """exp round for wave g: S_0 = exp(D/T), acc = row sums."""
            acc = small.tile([P, WV], f32)
            S = []
            for i in range(WV):
                t = g * WV + i
                s0 = gpool.tile([P, O], sdt, name="st")
                nc.scalar.activation(
                    s0[:, :],
                    dslab[:, t, :],
                    Act.Exp,
                    scale=invt[:, :],
                    bias=bias6[:, :],
                    accum_out=acc[:, i : i + 1],
                )
                S.append(s0)
            wave_state[g] = (S, acc)

        def emit_rounds(g):
            S, acc = wave_state.pop(g)
            outs = [outp.tile([P, O, K], odt, name="ot") for _ in range(WV)]
            for r in range(K):
                gam = small.tile([P, WV], f32)
                nc.vector.reciprocal(gam[:, :], acc[:, :])
                for i in range(WV):
                    f = outs[i][:, :, r]
                    gi = gam[:, i : i + 1]
                    e = pat[r * WV + i]
                    if e == "A":
                        nc.scalar.mul(f, S[i][:, :], gi)
                    elif e == "D":
                        nc.vector.tensor_scalar(f, S[i][:, :], gi, None, Alu.mult)
                    else:
                        nc.gpsimd.tensor_scalar(f, S[i][:, :], gi, None, Alu.mult)
                if r == 2 and g + 1 < WAVES:
                    # software pipeline: next wave's exps land on ACT now so
                    # its first reciprocal is ready at this wave's end.
                    emit_exps(g + 1)
                if r == K - 1:
                    break
                accn = small.tile([P, WV], f32)
                for i in range(WV):
                    gi = gam[:, i : i + 1]
                    if chain == "amr":
                        sn = gpool.tile([P, O], sdt, name="st")
                        nc.vector.affine_mul_reduce(
                            out=sn[:, :],
                            accum_out=accn[:, i : i + 1],
                            in0=S[i][:, :],
                            in1=S[i][:, :],
                            scale=gi,
                            bias=-1.0,
                        )
                    elif chain == "natr":
                        # native chain: S' = (S - a)*S is exact for any state
                        # scale; rescale keeps the state's squared-growth
                        # bounded (f32: once; fp16: twice, max |S| ~50).
                        rescale_rounds = (1, 4) if sdt == f16 else (2,)
                        if r in rescale_rounds:
                            fc = gpool.tile([P, O], sdt, name="st")
                            reng = nc.gpsimd if variant == "hybr3" else nc.vector
                            reng.tensor_scalar(
                                fc[:, :], S[i][:, :], gi, None, Alu.mult
                            )
                            src_t, scal = fc, 1.0
                        else:
                            src_t, scal = S[i], acc[:, i : i + 1]
                        sn = gpool.tile([P, O], sdt, name="st")
                        ceng = nc.vector
                        if variant == "mx1" and i % 4 != 0:
                            ceng = nc.gpsimd
                        ceng.scalar_tensor_tensor(
                            out=sn[:, :],
                            in0=src_t[:, :],
                            scalar=scal,
                            in1=src_t[:, :],
                            op0=Alu.subtract,
                            op1=Alu.mult,
                            accum_out=accn[:, i : i + 1],
                        )
                    else:
                        fc = gpool.tile([P, O], sdt, name="st")
                        nc.vector.tensor_scalar(
                            fc[:, :], S[i][:, :], gi, None, Alu.mult
                        )
                        sn = gpool.tile([P, O], sdt, name="st")
                        nc.vector.scalar_tensor_tensor(
                            out=sn[:, :],
                            in0=fc[:, :],
                            scalar=1.0,
                            in1=fc[:, :],
                            op0=Alu.subtract,
                            op1=Alu.mult,
                            accum_out=accn[:, i : i + 1],
                        )
                    S[i] = sn
                acc = accn
            if variant == "hybr2":
                # probe: stride-2 fp16 write cost (for pair-pack evaluation)
                pb = gpool.tile([P, O, 2], f16, name="pb", bufs=2)
                nc.vector.tensor_scalar(
                    pb[:, :, 0], S[0][:, :], gam[:, 0:1], None, Alu.mult
                )
                nc.vector.tensor_scalar(
                    pb[:, :, 1], S[1][:, :], gam[:, 1:2], None, Alu.mult
                )
                nc.scalar.mul(pb[:, :, 0], S[2][:, :], gam[:, 2:3])
            for i in range(WV):
                t = g * WV + i
                nc.sync.dma_start(
                    out=w[t * P : (t + 1) * P, :], in_=outs[i][:, :, :]
                )

        emit_exps(0)
        for g in range(WAVES):
            emit_rounds(g)

    nc.compile()
    return nc


def _get_nc(variant=None):
    variant = variant or VARIANT
    if variant not in _cached:
        _cached[variant] = _build(variant)
    return _cached[variant]


def _make_in_maps(D, log_temp):
    Dr = np.ascontiguousarray(np.asarray(D, dtype=np.float32).reshape(ROWS, O))
    lt = np.asarray(log_temp, dtype=np.float32).reshape(1, 1)
    return [
        {"d": Dr[c * RPC : (c + 1) * RPC], "log_temp": lt}
        for c in range(N_CORES)
    ]


def _gather(results):
    parts = [
        np.asarray(results[c]["w"], dtype=np.float32).reshape(RPC, O, K)
        for c in range(N_CORES)
    ]
    return np.concatenate(parts, axis=0).reshape(B, M, O, K)


def run_spmd(D, log_temp, trace=False, variant=None, **kwargs):
    """Run on all 8 cores; returns (W, BassKernelResults)."""
    from concourse.bass_utils import run_bass_kernel_spmd

    nc = _get_nc(variant)
    res = run_bass_kernel_spmd(
        nc, _make_in_maps(D, log_temp), list(range(N_CORES)), trace=trace, **kwargs
    )
    return _gather(res.results), res


def kernel(D, log_temp):
    W, _ = run_spmd(D, log_temp)
    return W


# revision 14
# speedup vs baseline: 2.3865x; 2.3865x over previous
"""Trainium2 Bass kernel: NeuralNearestNeighbors continuous-KNN weight volumes.

Reference computation (per row of D.reshape(b*m, o), K=8 rounds):
    logits = D / exp(log_temp)
    for k in range(K):
        w_k = log_softmax(logits);  out_k = exp(w_k)
        logits = logits + log1mexp(w_k)          # log(1 - p_k)
    W = stack(out_k, axis=-1)                     # (b, m, o, K)

Scale-invariant recurrence used on device: keep a state S that is an
arbitrary per-row scalar multiple of the round's softmax weights F_k,
with its true row-sum `a = sum(S)` tracked by the engines' accumulator.
Then F_k = S*(1/a) always (since sum(F_k) == 1), and

    S' = (F_k - 1) * F_k        a' = sum(S') = sum(F_k^2) - 1

reproduces the reference exactly:  F_{k+1} = (F_k - F_k^2)/(1 - sum F_k^2).
S stays in [-0.25, 0] after the first update, so fp16 state is safe
(all quantities carry *relative* fp16 error only).

Two chain variants:
  amr:    S' = (S*gam - 1)*S  via one custom-DVE affine_mul_reduce
          (gam = 1/a; equals (F-1)*F up to a positive-cancelling scale).
  native: Fc = S*gam (tensor_scalar), S' = (Fc-1)*Fc (scalar_tensor_tensor
          with accum) - two native DVE ops that can engage the 2-byte
          2x/4x DVE perf modes.

Per round each tile also writes F_k = S*gam into the k-strided slot of its
[P, O, K] output tile; those strided writes are spread over ACT/DVE/GpSimd
per a static pattern.  Recurrence rounds for a wave of 4 tiles share one
batched [P,4] reciprocal.  Waves are software-pipelined: the next wave's
exp's are emitted mid-wave so ACT never starves DVE at wave boundaries.

Output can be stored fp16 on device (halves HBM write traffic; rel err
~5e-4 << 2e-2 tolerance) and is upcast to f32 on the host.

Sharding: purely rowwise data-parallel over b*m = 16384 rows; 2048 rows
per core across 8 cores; log_temp replicated.
"""

import os

import numpy as np

B, M, O = 16, 1024, 512
K = 8
N_CORES = 8
ROWS = B * M                     # 16384
RPC = ROWS // N_CORES            # 2048 rows per core
P = 128
TILES = RPC // P                 # 16 row-tiles per core
WV = 4                           # tiles per wave
WAVES = TILES // WV

VARIANT = os.environ.get("KVAR", "h16a")

# variant -> (state_dtype, out_dtype, chain, pass1 engine counts (A, P, D))
_CFG = {
    "f32a": ("f32", "f32", "amr", (20, 6, 6)),
    "f32n": ("f32", "f32", "natr", (20, 6, 6)),
    "mx1": ("f32", "f32", "natr", (16, 0, 16)),
    "h16a": ("f16", "f16", "amr", (15, 13, 4)),
    "h16n": ("f16", "f16", "nat", (26, 0, 6)),
    "hyb": ("f16", "f32", "nat", (15, 0, 17)),
    "hybr": ("f16", "f32", "natr", (11, 0, 21)),
    "hybr2": ("f16", "f32", "natr", (23, 0, 9)),
    "hybr3": ("f16", "f32", "natr", (22, 0, 10)),
    "final": ("f16", "f32", "natr", (21, 0, 11)),
}

_cached = {}


def _make_pattern(n_act, n_pool, n_dve, total):
    """Largest-remainder round-robin spread of engine codes over slots."""
    pools = [("A", n_act), ("P", n_pool), ("D", n_dve)]
    credit = {c: 0.0 for c, _ in pools}
    out = []
    for _ in range(total):
        for c, n in pools:
            credit[c] += n / total
        pick = max(credit, key=lambda c: credit[c])
        credit[pick] -= 1.0
        out.append(pick)
    return out


def _build(variant):
    from contextlib import ExitStack

    import concourse.bacc as bacc
    import concourse.tile as tile
    from concourse import mybir

    f32 = mybir.dt.float32
    f16 = mybir.dt.float16
    Alu = mybir.AluOpType
    Act = mybir.ActivationFunctionType

    sdt_s, odt_s, chain, counts = _CFG[variant]
    sdt = f16 if sdt_s == "f16" else f32
    odt = f16 if odt_s == "f16" else f32
    pat = _make_pattern(*counts, total=K * WV)

    nc = bacc.Bacc(
        "TRN2",
        target_bir_lowering=False,
        debug=False,
        enable_asserts=False,
        num_devices=N_CORES,
    )
    d = nc.dram_tensor("d", [RPC, O], f32, kind="ExternalInput").ap()
    lt = nc.dram_tensor("log_temp", [1, 1], f32, kind="ExternalInput").ap()
    w = nc.dram_tensor("w", [RPC, O * K], odt, kind="ExternalOutput").ap()

    with tile.TileContext(nc) as tc, ExitStack() as ctx:
        singles = ctx.enter_context(tc.tile_pool(name="singles", bufs=1))
        dpool = ctx.enter_context(tc.tile_pool(name="dslab", bufs=1))
        gpool = ctx.enter_context(
            tc.tile_pool(name="state", bufs=28 if chain in ("nat", "natr") else 16)
        )
        outp = ctx.enter_context(
            tc.tile_pool(name="out", bufs=10 if odt == f16 else 7)
        )
        small = ctx.enter_context(tc.tile_pool(name="small", bufs=64))

        # log_temp -> 1/T = exp(-log_temp), replicated to all 128 partitions.
        lt_sb = singles.tile([P, 1], f32)
        nc.sync.dma_start(out=lt_sb[:, :], in_=lt.to_broadcast((P, 1)))
        invt = singles.tile([P, 1], f32)
        nc.scalar.activation(invt[:, :], lt_sb[:, :], Act.Exp, scale=-1.0)
        bias6 = singles.tile([P, 1], f32)
        nc.vector.memset(bias6[:, :], -6.0)

        din = d.rearrange("(t p) o -> p t o", p=P)
        dslab = dpool.tile([P, TILES, O], f32)
        for g in range(WAVES):
            # SWDGE path keeps the HWDGE rings free for output writes.
            nc.gpsimd.dma_start(
                out=dslab[:, g * WV : (g + 1) * WV, :],
                in_=din[:, g * WV : (g + 1) * WV, :],
            )

        wave_state = {}

        def emit_exps(g):
            """exp round for wave g: S_0 = exp(D/T), acc = row sums."""
            acc = small.tile([P, WV], f32)
            S = []
            for i in range(WV):
                t = g * WV + i
                s0 = gpool.tile([P, O], sdt, name="st")
                nc.scalar.activation(
                    s0[:, :],
                    dslab[:, t, :],
                    Act.Exp,
                    scale=invt[:, :],
                    bias=bias6[:, :],
                    accum_out=acc[:, i : i + 1],
                )
                S.append(s0)
            wave_state[g] = (S, acc)

        def emit_rounds(g):
            S, acc = wave_state.pop(g)
            outs = [outp.tile([P, O, K], odt, name="ot") for _ in range(WV)]
            for r in range(K):
                gam = small.tile([P, WV], f32)
                nc.vector.reciprocal(gam[:, :], acc[:, :])
                for i in range(WV):
                    f = outs[i][:, :, r]
                    gi = gam[:, i : i + 1]
                    e = pat[r * WV + i]
                    if e == "A":
                        nc.scalar.mul(f, S[i][:, :], gi)
                    elif e == "D":
                        nc.vector.tensor_scalar(f, S[i][:, :], gi, None, Alu.mult)
                    else:
                        nc.gpsimd.tensor_scalar(f, S[i][:, :], gi, None, Alu.mult)
                if r == 2 and g + 1 < WAVES:
                    # software pipeline: next wave's exps land on ACT now so
                    # its first reciprocal is ready at this wave's end.
                    emit_exps(g + 1)
                if r == K - 1:
                    break
                accn = small.tile([P, WV], f32)
                for i in range(WV):
                    gi = gam[:, i : i + 1]
                    if chain == "amr":
                        sn = gpool.tile([P, O], sdt, name="st")
                        nc.vector.affine_mul_reduce(
                            out=sn[:, :],
                            accum_out=accn[:, i : i + 1],
                            in0=S[i][:, :],
                            in1=S[i][:, :],
                            scale=gi,
                            bias=-1.0,
                        )
                    elif chain == "natr":
                        # native chain: S' = (S - a)*S is exact for any state
                        # scale; rescale keeps the state's squared-growth
                        # bounded (f32: once; fp16: twice, max |S| ~50).
                        rescale_rounds = (1, 4) if sdt == f16 else (2,)
                        if r in rescale_rounds:
                            fc = gpool.tile([P, O], sdt, name="st")
                            reng = nc.vector
                            reng.tensor_scalar(
                                fc[:, :], S[i][:, :], gi, None, Alu.mult
                            )
                            src_t, scal = fc, 1.0
                        else:
                            src_t, scal = S[i], acc[:, i : i + 1]
                        sn = gpool.tile([P, O], sdt, name="st")
                        ceng = nc.vector
                        if variant == "mx1" and i % 4 != 0:
                            ceng = nc.gpsimd
                        ceng.scalar_tensor_tensor(
                            out=sn[:, :],
                            in0=src_t[:, :],
                            scalar=scal,
                            in1=src_t[:, :],
                            op0=Alu.subtract,
                            op1=Alu.mult,
                            accum_out=accn[:, i : i + 1],
                        )
                    else:
                        fc = gpool.tile([P, O], sdt, name="st")
                        nc.vector.tensor_scalar(
                            fc[:, :], S[i][:, :], gi, None, Alu.mult
                        )
                        sn = gpool.tile([P, O], sdt, name="st")
                        nc.vector.scalar_tensor_tensor(
                            out=sn[:, :],
                            in0=fc[:, :],
                            scalar=1.0,
                            in1=fc[:, :],
                            op0=Alu.subtract,
                            op1=Alu.mult,
                            accum_out=accn[:, i : i + 1],
                        )
                    S[i] = sn
                acc = accn
            if variant == "probe":
                # probe: stride-2 fp16 write cost (for pair-pack evaluation)
                pb = gpool.tile([P, O, 2], f16, name="pb", bufs=2)
                nc.vector.tensor_scalar(
                    pb[:, :, 0], S[0][:, :], gam[:, 0:1], None, Alu.mult
                )
                nc.vector.tensor_scalar(
                    pb[:, :, 1], S[1][:, :], gam[:, 1:2], None, Alu.mult
                )
                nc.scalar.mul(pb[:, :, 0], S[2][:, :], gam[:, 2:3])
            for i in range(WV):
                t = g * WV + i
                nc.sync.dma_start(
                    out=w[t * P : (t + 1) * P, :], in_=outs[i][:, :, :]
                )

        emit_exps(0)
        for g in range(WAVES):
            emit_rounds(g)

    nc.compile()
    return nc


def _get_nc(variant=None):
    variant = variant or VARIANT
    if variant not in _cached:
        _cached[variant] = _build(variant)
    return _cached[variant]


def _make_in_maps(D, log_temp):
    Dr = np.ascontiguousarray(np.asarray(D, dtype=np.float32).reshape(ROWS, O))
    lt = np.asarray(log_temp, dtype=np.float32).reshape(1, 1)
    return [
        {"d": Dr[c * RPC : (c + 1) * RPC], "log_temp": lt}
        for c in range(N_CORES)
    ]


def _gather(results):
    parts = [
        np.asarray(results[c]["w"], dtype=np.float32).reshape(RPC, O, K)
        for c in range(N_CORES)
    ]
    return np.concatenate(parts, axis=0).reshape(B, M, O, K)


def run_spmd(D, log_temp, trace=False, variant=None, **kwargs):
    """Run on all 8 cores; returns (W, BassKernelResults)."""
    from concourse.bass_utils import run_bass_kernel_spmd

    nc = _get_nc(variant)
    res = run_bass_kernel_spmd(
        nc, _make_in_maps(D, log_temp), list(range(N_CORES)), trace=trace, **kwargs
    )
    return _gather(res.results), res


def kernel(D, log_temp):
    W, _ = run_spmd(D, log_temp)
    return W
